# revision 23
# baseline (speedup 1.0000x reference)
"""Trainium2 Bass kernel for a GQA attention block (RMSNorm -> QKV+gate ->
Q/K-norm -> RoPE -> attention -> gated out -> proj), tensor-parallel over
heads across 8 NeuronCores.

Sharding: core c owns q heads [5c, 5c+5) and kv group c (NQ=40, NKV=8).
Each core computes a partial projection output (bf16); partials are summed
on host (row-parallel proj unshard).

v2: bf16 matmul operands, QKV/proj weights SBUF-resident, x streamed once,
per-chunk software pipeline (stats/rope/v/gates of chunk c-1 emitted under
chunk c's QKV matmuls), deferred o-scale tails in attention so the in-order
PE queue never head-blocks on cross-engine chains.
"""
import sys

sys.path.insert(0, "/opt/trn_rl_repo")

import numpy as np

import bass_rust as _bass_rust

import concourse.bacc as bacc
import concourse.tile as tile
from concourse import mybir
from concourse.hw_specs import get_activation_tables


class _Bacc(bacc.Bacc):
    """Bacc with activation-table choice restricted to the exp+ln set
    (square/copy/exp/ln all live in one table -> zero table swaps)."""

    _KEEP_SETS = {"natural_log_exp_and_others", "sigmoid_and_others"}

    def insert_act_table_loads(self):
        has_activation = any(
            isinstance(i, mybir.InstActivation)
            for b in self.main_func.blocks
            for i in b.instructions
        )
        if not has_activation:
            return
        tables = [
            (name, (fns if name in self._KEEP_SETS else set()))
            for name, fns in get_activation_tables(self.m.arch).items()
        ]
        _bass_rust.insert_act_table_loads(self, tables)


NQ, NKV, D, HID = 40, 8, 128, 5120
S = 2048
NC = 8
HPC = NQ // NC          # q heads per core = 5
EPS = 1e-6
HT = HID // 128         # 40 hid tiles
KT = S // 128           # 16 k-tiles
CW = 256                # chunk width (seq)
NCH = S // CW           # 8 chunks
QKV_COLS = HPC * D + 2 * D + 128   # 1024 (gate padded 5 -> 128)
F32 = mybir.dt.float32
F32R = mybir.dt.float32r
BF16 = mybir.dt.bfloat16
AF = mybir.ActivationFunctionType
BUILD_OPTS = {}


def build_program(repeat=1):
    opt = BUILD_OPTS
    nc = _Bacc(None, target_bir_lowering=False)

    for val in (EPS, float(D) * EPS):
        t = nc.alloc_sbuf_tensor(f"const-float32-{val}", [128, 1], F32)
        nc.gpsimd.memset(t.ap(), val)
        nc.const_aps.aps[(F32, val)] = t.ap()
    nc.all_engine_barrier()

    # ---- I/O ----
    # x packed so one hq-group load is 2KB contiguous per partition
    xq = nc.dram_tensor("xq", [128, NCH, HT // 4, 4 * CW], BF16,
                        kind="ExternalInput")
    wqd = nc.dram_tensor("wqd", [128, HT, QKV_COLS], BF16, kind="ExternalInput")
    wpd = nc.dram_tensor("wpd", [128, HPC, HID], BF16, kind="ExternalInput")
    cosq = nc.dram_tensor("cosq", [128, S], BF16, kind="ExternalInput")
    sinq = nc.dram_tensor("sinq", [128, S], BF16, kind="ExternalInput")
    cosk = nc.dram_tensor("cosk", [128, S], BF16, kind="ExternalInput")
    sink = nc.dram_tensor("sink", [128, S], BF16, kind="ExternalInput")
    ones_col = nc.dram_tensor("ones_col", [128, 1], F32R, kind="ExternalInput")
    ones_bf = nc.dram_tensor("ones_bf", [128, 1], BF16, kind="ExternalInput")
    ones_row = nc.dram_tensor("ones_row", [1, 128], F32R, kind="ExternalInput")
    ident = nc.dram_tensor("ident", [128, 128], F32R, kind="ExternalInput")
    out = nc.dram_tensor("out", [S, HID], BF16, kind="ExternalOutput")
    if opt.get("debug"):
        dbg_q0 = nc.dram_tensor("dbg_q0", [128, S], BF16,
                                kind="ExternalOutput")
        dbg_k = nc.dram_tensor("dbg_k", [128, S], BF16, kind="ExternalOutput")
        dbg_v = nc.dram_tensor("dbg_v", [128, S], BF16, kind="ExternalOutput")
        dbg_g = nc.dram_tensor("dbg_g", [HPC, S], F32, kind="ExternalOutput")
        dbg_o0 = nc.dram_tensor("dbg_o0", [128, S], BF16,
                                kind="ExternalOutput")

    with tile.TileContext(nc, pool_alloc_mode=opt.get("palloc", "stack")) as tc:
      for _rep in range(repeat):
        with tc.tile_pool(name=f"persist{_rep}", bufs=1) as pers, \
             tc.tile_pool(name=f"scr{_rep}", bufs=1, space="DRAM") as dscr:
            # DRAM row scratch (partition reshapes / broadcasts)
            rrow_scr = dscr.tile([1, S], F32, name="rrow_scr")
            nk_scr = dscr.tile([1, S], F32, name="nk_scr")
            nq_scr = dscr.tile([HPC, S], F32, name="nq_scr")
            gate_scr = dscr.tile([HPC, S], F32, name="gate_scr")
            den_scr = dscr.tile([2 * HPC, 1024], F32, name="den_scr")

            # persistent tiles
            t_ones = pers.tile([128, 1], F32R, name="ones")
            nc.sync.dma_start(t_ones[:, :], ones_col[:, :])
            t_onesb = pers.tile([128, 1], BF16, name="onesb")
            nc.sync.dma_start(t_onesb[:, :], ones_bf[:, :])
            t_onesr = pers.tile([1, 128], F32R, name="onesr")
            nc.sync.dma_start(t_onesr[:, :], ones_row[:, :])
            t_id = pers.tile([128, 128], F32R, name="ident")
            nc.sync.dma_start(t_id[:, :], ident[:, :])

            q_t = [pers.tile([128, S], BF16, name=f"q{h}") for h in range(HPC)]
            k_t = pers.tile([128, S], BF16, name="kT")
            vnat = pers.tile([128, S], BF16, name="vnat")

            # ============ Stage 1: QKV + per-chunk post pipeline ============
            with tc.tile_pool(name=f"wq{_rep}", bufs=1) as wqp, \
                 tc.tile_pool(name=f"rt{_rep}", bufs=1) as rtp, \
                 tc.tile_pool(name=f"s1ps{_rep}", bufs=1, space="PSUM") as psA, \
                 tc.tile_pool(name=f"s1row{_rep}", bufs=3, space="PSUM") as psR, \
                 tc.tile_pool(name=f"s1tr{_rep}", bufs=1, space="PSUM") as psT, \
                 tc.tile_pool(name=f"s1x{_rep}", bufs=4) as sbx, \
                 tc.tile_pool(name=f"s1acc{_rep}", bufs=2) as sba, \
                 tc.tile_pool(name=f"s1sq{_rep}", bufs=2) as sbq, \
                 tc.tile_pool(name=f"s1d{_rep}", bufs=2) as sbd, \
                 tc.tile_pool(name=f"s1r{_rep}", bufs=2) as sbr:
                # resident weights (pieces interleaved with chunk-0 x DMAs)
                wq = wqp.tile([128, HT, QKV_COLS], BF16, name="wq")
                tcq = rtp.tile([128, S], BF16, name="tcq")
                tsq = rtp.tile([128, S], BF16, name="tsq")
                tck = rtp.tile([128, S], BF16, name="tck")
                tsk = rtp.tile([128, S], BF16, name="tsk")

                psum_state = {}
                sq_state = {}

                def emit_qkv(ch):
                    pm = psA.tile([128, 8, CW], F32, name="pm")
                    acc4 = sba.tile([128, 1024], F32, name="acc4")
                    NQU = HT // 4
                    for hq in range(NQU):
                        xt = sbx.tile([128, 4 * CW], BF16, name="xt")
                        nc.sync.dma_start(xt[:, :], xq[:, ch, hq, :])
                        if ch == 0 and hq < 6:
                            # each piece must be emitted no later than the
                            # first matmul that reads it (emission order IS
                            # the dependency order for the tile framework).
                            # Weight DMAs ride the Scalar queue so they don't
                            # delay x tiles on SP.
                            pieces = [[(0, 1), (1, 2), (2, 4)], [(4, 8)],
                                      [(8, 16)], [(16, 24)], [(24, 32)],
                                      [(32, 40)]]
                            for a, b in pieces[hq]:
                                nc.scalar.dma_start(wq[:, a:b, :],
                                                    wqd[:, a:b, :])
                        if ch == 0 and hq == 5:
                            nc.scalar.dma_start(tcq[:, :], cosq[:, :])
                            nc.scalar.dma_start(tsq[:, :], sinq[:, :])
                            nc.scalar.dma_start(tck[:, :], cosk[:, :])
                            nc.scalar.dma_start(tsk[:, :], sink[:, :])
                        for i in range(4):
                            ht = hq * 4 + i
                            # PSUM has_written clear is BANK-granular: only
                            # the even (first-emitted) slice of each 2KB bank
                            # may set start; its odd partner's first write
                            # lands on cleared bits and sets them itself.
                            st = (ht == 0)
                            sp = (ht == HT - 1)
                            for m in range(8):
                                nc.tensor.matmul(
                                    pm[:, m, :],
                                    wq[:, ht, m * 128:(m + 1) * 128],
                                    xt[:, i * CW:(i + 1) * CW],
                                    start=(st and m % 2 == 0),
                                    stop=sp)
                        sqx = sbq.tile([128, 4 * CW], F32, name="sqx")
                        nc.scalar.activation(sqx[:, :], xt[:, :], AF.Square)
                        if hq == 0:
                            nc.gpsimd.tensor_copy(acc4[:, :], sqx[:, :])
                        else:
                            nc.gpsimd.tensor_add(acc4[:, :], acc4[:, :],
                                                 sqx[:, :])
                    psum_state[ch] = (pm, acc4)

                def drain_copies(ch):
                    """Free psA quickly (chunk ch+1's matmuls WAR on pm).
                    Nothing on Scalar: its queue must reach post_stats'
                    row Lns fast -- those free the psR tiles the next
                    stats matmuls (in-order PE queue) wait on."""
                    pm, acc4 = psum_state.pop(ch)
                    c0 = ch * CW
                    nc.vector.tensor_copy(k_t[:, c0:c0 + CW], pm[:, 0, :])
                    for h in range(3):
                        nc.vector.tensor_copy(q_t[h][:, c0:c0 + CW],
                                              pm[:, 1 + h, :])
                    for h in range(3, HPC):
                        nc.vector.tensor_copy(q_t[h][:, c0:c0 + CW],
                                              pm[:, 1 + h, :])
                    vtmp = sbd.tile([128, CW], F32R, name="vtmp")
                    nc.vector.tensor_copy(vtmp[:, :], pm[:, 6, :])
                    graw = sbd.tile([5, CW], F32, name="graw")
                    nc.vector.tensor_copy(graw[:, :], pm[0:5, 7, :])
                    sq_state[ch] = [acc4, vtmp, graw]

                def post_squares(ch):
                    """Squares for the deferred stats matmuls. Emitted AFTER
                    post_stats(ch-1) so the stats Ln/Exp row ops sit ahead of
                    these in the Scalar queue (the row Lns free the psR psum
                    tiles the next stats matmuls need)."""
                    acc4, vtmp, graw = sq_state[ch]
                    c0 = ch * CW
                    accr = sbd.tile([128, CW], BF16, name="accr")
                    with nc.allow_low_precision(
                            reason="sum(x^2) ~ 5e3; bf16 rel err 4e-3 ok"):
                        nc.vector.tensor_reduce(
                            accr[:, :],
                            acc4[:, :].rearrange("p (q s) -> p s q", q=4),
                            axis=mybir.AxisListType.X, op=mybir.AluOpType.add)
                    ksq = sbd.tile([128, CW], BF16, name="ksq")
                    nc.scalar.activation(ksq[:, :], k_t[:, c0:c0 + CW],
                                         AF.Square)
                    qsq = sbd.tile([128, HPC, CW], BF16, name="qsq")
                    for h in range(HPC):
                        nc.scalar.activation(qsq[:, h, :],
                                             q_t[h][:, c0:c0 + CW], AF.Square)
                    sq_state[ch] = (accr, ksq, qsq, vtmp, graw)

                def post_stats(ch):
                    """Stats matmuls + rows + broadcasts + rope + v + gates."""
                    accr, ksq, qsq, vtmp, graw = sq_state.pop(ch)
                    c0 = ch * CW
                    # ---- stat rows (Act chain has no DMA round-trips) ----
                    pr = psR.tile([1, CW], F32, name="row")
                    nc.tensor.matmul(pr[:, :], t_onesb[:, :], accr[:, :],
                                     start=True, stop=True)
                    lnm_row = sbr.tile([1, CW], F32, name="lnm_row")
                    nc.scalar.activation(lnm_row[:, :], pr[:, :], AF.Ln,
                                         bias=EPS, scale=1.0 / HID)
                    r_row = sbr.tile([1, CW], F32, name="r_row")
                    nc.scalar.activation(r_row[:, :], lnm_row[:, :], AF.Exp,
                                         bias=0.0, scale=-0.5)
                    nc.sync.dma_start(rrow_scr[0:1, c0:c0 + CW], r_row[:, :])

                    pn = psR.tile([1, CW], F32, name="row")
                    nc.tensor.matmul(pn[:, :], t_onesb[:, :], ksq[:, :],
                                     start=True, stop=True)
                    lnk_row = sbr.tile([1, CW], F32, name="lnk_row")
                    nc.scalar.activation(lnk_row[:, :], pn[:, :], AF.Ln,
                                         bias=D * EPS, scale=1.0)
                    nkr = sbr.tile([1, CW], F32, name="nkr")
                    nc.scalar.activation(nkr[:, :], lnk_row[:, :], AF.Exp,
                                         bias=0.0, scale=-0.5)
                    nc.sync.dma_start(nk_scr[0:1, c0:c0 + CW], nkr[:, :])

                    for h in range(HPC):
                        pq = psR.tile([1, CW], F32, name="row")
                        nc.tensor.matmul(pq[:, :], t_onesb[:, :], qsq[:, h, :],
                                         start=True, stop=True)
                        lnq = sbr.tile([1, CW], F32, name="lnq")
                        nc.scalar.activation(lnq[:, :], pq[:, :], AF.Ln,
                                             bias=EPS, scale=1.0 / D)
                        nqr = sbr.tile([1, CW], F32, name="nqr")
                        nc.scalar.activation(nqr[:, :], lnq[:, :],
                                             AF.Exp, bias=0.0, scale=-0.5)
                        nc.sync.dma_start(nq_scr[h:h + 1, c0:c0 + CW],
                                          nqr[:, :])

                    # ---- broadcasts ----
                    rbig = sbr.tile([128, CW], F32, name="rbig")
                    nc.sync.dma_start(
                        rbig[:, :],
                        rrow_scr[0:1, c0:c0 + CW].to_broadcast((128, CW)))
                    nkb = sbr.tile([128, CW], F32, name="nkb")
                    nc.sync.dma_start(
                        nkb[:, :],
                        nk_scr[0:1, c0:c0 + CW].to_broadcast((128, CW)))
                    rb = sbr.tile([5, CW], F32, name="rb")
                    nc.sync.dma_start(
                        rb[:, :],
                        rrow_scr[0:1, c0:c0 + CW].to_broadcast((5, CW)))

                    # ---- v: scale by r then transpose ----
                    nc.vector.tensor_mul(vtmp[:, :], vtmp[:, :], rbig[:, :])
                    for j in range(2):
                        kt = 2 * ch + j
                        ptr = psT.tile([128, 128], F32R, name="tr")
                        nc.tensor.transpose(ptr[:, :],
                                            vtmp[:, j * 128:(j + 1) * 128],
                                            t_id[:, :])
                        nc.vector.tensor_copy(
                            vnat[:, kt * 128:(kt + 1) * 128], ptr[:, :])

                    # ---- k rope + fold k-norm (exp scale becomes 1) ----
                    rot = sbr.tile([128, CW], BF16, name="rotk")
                    nc.sync.dma_start(rot[0:64, :], k_t[64:128, c0:c0 + CW])
                    nc.sync.dma_start(rot[64:128, :], k_t[0:64, c0:c0 + CW])
                    t1 = sbr.tile([128, CW], F32, name="t1k")
                    nc.vector.tensor_mul(t1[:, :], k_t[:, c0:c0 + CW],
                                         tck[:, c0:c0 + CW])
                    t2 = sbr.tile([128, CW], F32, name="t2k")
                    nc.vector.tensor_mul(t2[:, :], rot[:, :],
                                         tsk[:, c0:c0 + CW])
                    nc.vector.tensor_add(t1[:, :], t1[:, :], t2[:, :])
                    nc.vector.tensor_mul(k_t[:, c0:c0 + CW], t1[:, :],
                                         nkb[:, :])

                    # ---- q rope per head ----
                    for h in range(HPC):
                        nb = sbr.tile([128, CW], F32, name="nb")
                        nc.sync.dma_start(
                            nb[:, :],
                            nq_scr[h:h + 1, c0:c0 + CW].to_broadcast((128, CW)))
                        rotq = sbr.tile([128, CW], BF16, name="rotq")
                        nc.sync.dma_start(rotq[0:64, :],
                                          q_t[h][64:128, c0:c0 + CW])
                        nc.sync.dma_start(rotq[64:128, :],
                                          q_t[h][0:64, c0:c0 + CW])
                        tq1 = sbr.tile([128, CW], F32, name="tq1")
                        nc.vector.tensor_mul(tq1[:, :], q_t[h][:, c0:c0 + CW],
                                             tcq[:, c0:c0 + CW])
                        tq2 = sbr.tile([128, CW], F32, name="tq2")
                        nc.vector.tensor_mul(tq2[:, :], rotq[:, :],
                                             tsq[:, c0:c0 + CW])
                        nc.vector.tensor_add(tq1[:, :], tq1[:, :], tq2[:, :])
                        nc.vector.tensor_mul(q_t[h][:, c0:c0 + CW], tq1[:, :],
                                             nb[:, :])

                    # ---- gates: store softplus(-x) = -ln(sigmoid(x)).
                    # The tail computes exp(-(ln denom + sp)) = gate/denom,
                    # so no reciprocal is ever needed (DVE reciprocal is
                    # 8 cycles/element -- 2.1us per [5,CW] row).
                    nc.vector.tensor_mul(graw[:, :], graw[:, :], rb[:, :])
                    ge = sbr.tile([5, CW], F32, name="ge")
                    nc.scalar.activation(ge[:, :], graw[:, :], AF.Exp,
                                         bias=0.0, scale=-1.0)
                    nc.vector.tensor_scalar_add(ge[:, :], ge[:, :], 1.0)
                    sp = sbr.tile([5, CW], F32, name="sp")
                    nc.scalar.activation(sp[:, :], ge[:, :], AF.Ln)
                    nc.sync.dma_start(gate_scr[0:5, c0:c0 + CW], sp[:, :])

                for ch in range(NCH):
                    emit_qkv(ch)
                    # drain first: frees the 4 psA banks for chunk ch+1's
                    # matmuls before anything else queues up on Scalar.
                    drain_copies(ch)
                    if ch >= 1:
                        post_stats(ch - 1)
                    post_squares(ch)
                post_stats(NCH - 1)

            if opt.get("debug"):
                nc.sync.dma_start(dbg_q0[:, :], q_t[0][:, :])
                nc.sync.dma_start(dbg_k[:, :], k_t[:, :])
                nc.sync.dma_start(dbg_v[:, :], vnat[:, :])
                nc.sync.dma_start(dbg_g[:, :], gate_scr[:, :])

            # ============ Stages 2+3 ============
            with tc.tile_pool(name=f"wp{_rep}", bufs=1) as wpp:
                # prefetch proj weights while attention runs
                wp = wpp.tile([128, HPC, HID], BF16, name="wp")
                nc.sync.dma_start(wp[:, :, :], wpd[:, :, :])
                o_t = [wpp.tile([128, S], BF16, name=f"o{h}")
                       for h in range(HPC)]

                # ---- Stage 2: attention ----
                with tc.tile_pool(name=f"at_sc{_rep}", bufs=2, space="PSUM") as pSC, \
                     tc.tile_pool(name=f"at_av{_rep}", bufs=1, space="PSUM") as pAV, \
                     tc.tile_pool(name=f"at_row{_rep}", bufs=1, space="PSUM") as pRow, \
                     tc.tile_pool(name=f"at_acc{_rep}", bufs=1) as asb1, \
                     tc.tile_pool(name=f"at_sb{_rep}", bufs=3) as asb2, \
                     tc.tile_pool(name=f"at_et{_rep}", bufs=5) as asb3:
                    pending_tail = []

                    def emit_core(h, qp):
                        c0 = qp * 1024
                        po = pAV.tile([128, 1024], F32, name="po")
                        accA = asb1.tile([128, 1024], BF16, name="accA")
                        accB = asb1.tile([128, 1024], BF16, name="accB")
                        accC = asb1.tile([128, 1024], BF16, name="accC",
                                         bufs=2)
                        ps_tiles = {}

                        def emit_sc(kt):
                            ps = pSC.tile([128, 1024], F32, name="sc")
                            for j in range(2):
                                nc.tensor.matmul(
                                    ps[:, j * 512:(j + 1) * 512],
                                    k_t[:, kt * 128:(kt + 1) * 128],
                                    q_t[h][:, c0 + j * 512:c0 + (j + 1) * 512],
                                    start=True, stop=True)
                            ps_tiles[kt] = ps

                        emit_sc(0)
                        for kt in range(KT):
                            k0 = kt * 128
                            if kt + 1 < KT:
                                emit_sc(kt + 1)
                            ps = ps_tiles.pop(kt)
                            et = asb3.tile([128, 1024], BF16, name="expt")
                            nc.scalar.activation(et[:, :], ps[:, :], AF.Exp)
                            for j in range(2):
                                nc.tensor.matmul(
                                    po[:, j * 512:(j + 1) * 512],
                                    vnat[:, k0:k0 + 128],
                                    et[:, j * 512:(j + 1) * 512],
                                    start=(kt == 0), stop=(kt == KT - 1))
                            use_pool = h > 0 and (kt == 0 or kt % 4 == 0)
                            with nc.allow_low_precision(
                                    reason="softmax denom; 2e-2 tolerance"):
                                if kt == 0 and use_pool:
                                    nc.gpsimd.tensor_copy(accA[:, :], et[:, :])
                                elif kt == 0:
                                    nc.vector.tensor_copy(accA[:, :], et[:, :])
                                elif kt == 1:
                                    nc.vector.tensor_copy(accB[:, :], et[:, :])
                                elif use_pool:
                                    nc.gpsimd.tensor_add(accA[:, :],
                                                         accA[:, :], et[:, :])
                                else:
                                    nc.vector.tensor_add(accB[:, :],
                                                         accB[:, :], et[:, :])
                        with nc.allow_low_precision(
                                reason="softmax denom; 2e-2 tolerance"):
                            nc.vector.tensor_add(accC[:, :], accA[:, :],
                                                 accB[:, :])
                        # drain AV psum to o_t (unscaled); split engines so
                        # both po banks free in parallel
                        nc.vector.tensor_copy(o_t[h][:, c0:c0 + 512],
                                              po[:, 0:512])
                        nc.scalar.copy(o_t[h][:, c0 + 512:c0 + 1024],
                                       po[:, 512:1024])
                        return accC

                    def emit_tail(h, qp, accC):
                        # o_t scale = gate/denom per q column. Row reduce on
                        # PE, ln on the Scalar row pipe (rows are full speed
                        # on ACT, ~6x slower per element on DVE), broadcast
                        # via DRAM round-trip, exp(-x) full-width, then two
                        # DVE muls. Deferred one block: nothing here stalls
                        # the PE queue.
                        c0 = qp * 1024
                        b = h * 2 + qp
                        prow = pRow.tile([1, 1024], F32, name="drow")
                        for j in range(2):
                            nc.tensor.matmul(prow[0:1, j * 512:(j + 1) * 512],
                                             t_onesb[:, :],
                                             accC[:, j * 512:(j + 1) * 512],
                                             start=True, stop=True)
                        lnr = asb2.tile([1, 1024], F32, name="lnr", bufs=2)
                        nc.scalar.activation(lnr[:, :], prow[0:1, :], AF.Ln)
                        nc.sync.dma_start(den_scr[b:b + 1, :], lnr[:, :])
                        lnb = asb2.tile([128, 1024], F32, name="lnb", bufs=2)
                        nc.sync.dma_start(
                            lnb[:, :],
                            den_scr[b:b + 1, :].to_broadcast((128, 1024)))
                        gab = asb2.tile([128, 1024], F32, name="gab", bufs=2)
                        nc.sync.dma_start(
                            gab[:, :],
                            gate_scr[h:h + 1, c0:c0 + 1024].to_broadcast(
                                (128, 1024)))
                        # gate_scr holds -ln(gate); exp(-(ln den + sp))
                        # = gate/denom in one activation
                        rcb = asb2.tile([128, 1024], F32, name="rcb", bufs=2)
                        nc.vector.tensor_add(lnb[:, :], lnb[:, :], gab[:, :])
                        nc.scalar.activation(rcb[:, :], lnb[:, :], AF.Exp,
                                             bias=0.0, scale=-1.0)
                        nc.vector.tensor_mul(o_t[h][:, c0:c0 + 1024],
                                             o_t[h][:, c0:c0 + 1024],
                                             rcb[:, :])

                    for h in range(HPC):
                        for qp in range(2):
                            accC = emit_core(h, qp)
                            pending_tail.append((h, qp, accC))
                            if len(pending_tail) > 1:
                                emit_tail(*pending_tail.pop(0))
                    while pending_tail:
                        emit_tail(*pending_tail.pop(0))

                if opt.get("debug"):
                    nc.sync.dma_start(dbg_o0[:, :], o_t[0][:, :])

                # ---- Stage 3: projection ----
                with tc.tile_pool(name=f"pj_ps{_rep}", bufs=3, space="PSUM") as pPJ, \
                     tc.tile_pool(name=f"pj_sb{_rep}", bufs=3) as pjs:
                    NTP = HID // 1024  # 5
                    for ntp in range(NTP):
                        n0 = ntp * 1024
                        for st in range(KT):
                            s0 = st * 128
                            pp = pPJ.tile([128, 1024], F32, name="pj")
                            # h outer / j inner: consecutive matmul pairs
                            # share the stationary (one LDWEIGHTS per pair)
                            # and alternate PSUM banks.
                            for h in range(HPC):
                                for j in range(2):
                                    nc.tensor.matmul(
                                        pp[:, j * 512:(j + 1) * 512],
                                        o_t[h][:, s0:s0 + 128],
                                        wp[:, h, n0 + j * 512:n0 + (j + 1) * 512],
                                        start=(h == 0), stop=(h == HPC - 1))
                            ob = pjs.tile([128, 1024], BF16, name="outsb")
                            if st % 2 == 0:
                                nc.vector.tensor_copy(ob[:, :], pp[:, :])
                            else:
                                nc.scalar.copy(ob[:, :], pp[:, :])
                            nc.sync.dma_start(out[s0:s0 + 128, n0:n0 + 1024],
                                              ob[:, :])
    nc.finalize()
    return nc


# ---------------- host-side prep & execution ----------------

_CACHE = {}


def _get_exec(repeat=1):
    key = (repeat, tuple(sorted(BUILD_OPTS.items())))
    if key in _CACHE:
        return _CACHE[key]

    import jax
    from concourse import bass2jax, mybir as mb
    from jax.experimental.shard_map import shard_map
    from jax.sharding import Mesh, PartitionSpec

    bass2jax.install_neuronx_cc_hook()
    nc = build_program(repeat)

    part_name = nc.partition_id_tensor.name if nc.partition_id_tensor else None
    in_names, out_names, out_avals = [], [], []
    for alloc in nc.m.functions[0].allocations:
        if not isinstance(alloc, mb.MemoryLocationSet):
            continue
        name = alloc.memorylocations[0].name
        if alloc.kind == "ExternalInput":
            if name != part_name:
                in_names.append(name)
        elif alloc.kind == "ExternalOutput":
            out_names.append(name)
            out_avals.append(jax.core.ShapedArray(tuple(alloc.tensor_shape),
                                                  mb.dt.np(alloc.dtype)))
    n_params = len(in_names)
    all_names = in_names + out_names
    if part_name is not None:
        all_names = all_names + [part_name]

    def _body(*args):
        operands = list(args)
        if part_name is not None:
            operands.append(bass2jax.partition_id_tensor())
        outs = bass2jax._bass_exec_p.bind(
            *operands,
            out_avals=tuple(out_avals),
            in_names=tuple(all_names),
            out_names=tuple(out_names),
            lowering_input_output_aliases=(),
            sim_require_finite=True,
            sim_require_nnan=True,
            nc=nc,
        )
        return tuple(outs)

    devices = jax.devices()[:NC]
    mesh = Mesh(np.asarray(devices), ("core",))
    spec = (PartitionSpec("core"),) * (n_params + len(out_names))
    # Donate the output-backing buffers: without donation XLA copies the
    # zero-init arrays into fresh output buffers every dispatch (~21MB/core).
    donate = tuple(range(n_params, n_params + len(out_names)))
    fn = jax.jit(shard_map(_body, mesh=mesh, in_specs=spec,
                           out_specs=(PartitionSpec("core"),) * len(out_names),
                           check_rep=False), keep_unused=True,
                 donate_argnums=donate)
    _CACHE[key] = dict(fn=fn, nc=nc, in_names=in_names, out_names=out_names,
                       out_avals=out_avals, mesh=mesh)
    return _CACHE[key]


def prep_inputs(x, rope_cos, rope_sin, w_pre_norm, w_qkv, w_q_norm, w_k_norm,
                w_proj):
    """Build the per-core input dict list (host-side sharding/layout only)."""
    import ml_dtypes
    bf16 = ml_dtypes.bfloat16

    x = np.asarray(x, np.float32)
    w_qkv = np.asarray(w_qkv, np.float32)
    w_proj = np.asarray(w_proj, np.float32)
    w_pre = np.asarray(w_pre_norm, np.float32)
    w_qn = np.asarray(w_q_norm, np.float32)
    w_kn = np.asarray(w_k_norm, np.float32)
    cos = np.asarray(rope_cos, np.float32)[0]   # [S, D]
    sin = np.asarray(rope_sin, np.float32)[0]

    # x: [S, HID] -> [128, NCH, HT//4, 4*CW] bf16 (hq-group contiguous)
    xT = x[0].T                                  # [HID, S]
    xqh = np.ascontiguousarray(
        xT.reshape(HT // 4, 4, 128, NCH, CW).transpose(2, 3, 0, 1, 4)
        .reshape(128, NCH, HT // 4, 4 * CW)).astype(bf16)

    cosT = np.ascontiguousarray(cos.T)          # [D, S]
    sinT = np.ascontiguousarray(sin.T)
    sign = np.where(np.arange(D) < D // 2, -1.0, 1.0).astype(np.float32)

    def rope_tables(w):
        w_swap = np.concatenate([w[D // 2:], w[:D // 2]])
        c = cosT * w[:, None]
        s = sinT * (sign * w_swap)[:, None]
        return np.ascontiguousarray(c), np.ascontiguousarray(s)

    cq, sq_ = rope_tables(w_qn)
    ck, sk = rope_tables(w_kn)
    cq, sq_, ck, sk = (a.astype(bf16) for a in (cq, sq_, ck, sk))

    wqkv_eff = w_pre[:, None] * w_qkv           # fold pre-norm weight (exact)
    q_dim, k_dim = NQ * D, NKV * D
    ones = np.ones((128, 1), np.float32)
    ones_b = np.ones((128, 1), bf16)
    ones_r = np.ones((1, 128), np.float32)
    ident = np.eye(128, dtype=np.float32)

    gate_pad = np.zeros((HID, 128 - HPC), np.float32)
    in_maps = []
    for c in range(NC):
        wslice = np.concatenate([
            wqkv_eff[:, q_dim + c * D:q_dim + (c + 1) * D],
            wqkv_eff[:, (HPC * c) * D:(HPC * c + HPC) * D],
            wqkv_eff[:, q_dim + k_dim + c * D:q_dim + k_dim + (c + 1) * D],
            wqkv_eff[:, q_dim + 2 * k_dim + HPC * c:q_dim + 2 * k_dim + HPC * (c + 1)],
            gate_pad,
        ], axis=1)                               # [HID, 1024] (k first)
        wqd = np.ascontiguousarray(
            wslice.reshape(HT, 128, QKV_COLS).transpose(1, 0, 2)).astype(bf16)
        wpd = np.ascontiguousarray(
            w_proj[(HPC * c) * D:(HPC * c + HPC) * D, :].reshape(
                HPC, 128, HID).transpose(1, 0, 2)).astype(bf16)
        in_maps.append({
            "xq": xqh, "wqd": wqd, "wpd": wpd,
            "cosq": cq, "sinq": sq_, "cosk": ck, "sink": sk,
            "ones_col": ones, "ones_bf": ones_b, "ones_row": ones_r,
            "ident": ident,
        })
    return in_maps


def run_in_maps(in_maps):
    """Execute the SPMD program; returns list of per-core {out: [S, HID]}."""
    cache = _get_exec()
    fn, in_names, out_names, out_avals = (cache["fn"], cache["in_names"],
                                          cache["out_names"], cache["out_avals"])
    concat_in = [np.concatenate([m[nm] for m in in_maps], axis=0)
                 for nm in in_names]
    zeros = [np.zeros((NC * a.shape[0], *a.shape[1:]), a.dtype) for a in out_avals]
    outs = fn(*concat_in, *zeros)
    res = []
    for c in range(NC):
        d = {}
        for i, nm in enumerate(out_names):
            shp = out_avals[i].shape
            d[nm] = np.asarray(outs[i]).reshape(NC, *shp)[c]
        res.append(d)
    return res


def kernel(**inputs):
    in_maps = prep_inputs(**inputs)
    res = run_in_maps(in_maps)
    total = res[0]["out"].astype(np.float32)
    for c in range(1, NC):
        total = total + res[c]["out"].astype(np.float32)
    return total.reshape(1, S, HID)



# revision 28
# speedup vs baseline: 1.0380x; 1.0380x over previous
"""Trainium2 Bass kernel for a GQA attention block (RMSNorm -> QKV+gate ->
Q/K-norm -> RoPE -> attention -> gated out -> proj), tensor-parallel over
heads across 8 NeuronCores.

Sharding: core c owns q heads [5c, 5c+5) and kv group c (NQ=40, NKV=8).
Each core computes a partial projection output (bf16); partials are summed
on host (row-parallel proj unshard).

v2: bf16 matmul operands, QKV/proj weights SBUF-resident, x streamed once,
per-chunk software pipeline (stats/rope/v/gates of chunk c-1 emitted under
chunk c's QKV matmuls), deferred o-scale tails in attention so the in-order
PE queue never head-blocks on cross-engine chains.
"""
import sys

sys.path.insert(0, "/opt/trn_rl_repo")

import numpy as np

import bass_rust as _bass_rust

import concourse.bacc as bacc
import concourse.tile as tile
from concourse import mybir
from concourse.hw_specs import get_activation_tables


class _Bacc(bacc.Bacc):
    """Bacc with activation-table choice restricted to the exp+ln set
    (square/copy/exp/ln all live in one table -> zero table swaps)."""

    _KEEP_SETS = {"natural_log_exp_and_others", "sigmoid_and_others"}

    def insert_act_table_loads(self):
        has_activation = any(
            isinstance(i, mybir.InstActivation)
            for b in self.main_func.blocks
            for i in b.instructions
        )
        if not has_activation:
            return
        tables = [
            (name, (fns if name in self._KEEP_SETS else set()))
            for name, fns in get_activation_tables(self.m.arch).items()
        ]
        _bass_rust.insert_act_table_loads(self, tables)


NQ, NKV, D, HID = 40, 8, 128, 5120
S = 2048
NC = 8
HPC = NQ // NC          # q heads per core = 5
EPS = 1e-6
HT = HID // 128         # 40 hid tiles
KT = S // 128           # 16 k-tiles
CW = 256                # chunk width (seq)
NCH = S // CW           # 8 chunks
QKV_COLS = HPC * D + 2 * D + 128   # 1024 (gate padded 5 -> 128)
F32 = mybir.dt.float32
F32R = mybir.dt.float32r
BF16 = mybir.dt.bfloat16
AF = mybir.ActivationFunctionType
BUILD_OPTS = {}


def build_program(repeat=1):
    opt = BUILD_OPTS
    nc = _Bacc(None, target_bir_lowering=False)

    for val in (EPS, float(D) * EPS):
        t = nc.alloc_sbuf_tensor(f"const-float32-{val}", [128, 1], F32)
        nc.gpsimd.memset(t.ap(), val)
        nc.const_aps.aps[(F32, val)] = t.ap()
    nc.all_engine_barrier()

    # ---- I/O ----
    # x packed so one hq-group load is 2KB contiguous per partition
    xq = nc.dram_tensor("xq", [128, NCH, HT // 4, 4 * CW], BF16,
                        kind="ExternalInput")
    wqd = nc.dram_tensor("wqd", [128, 8, HT, 128], BF16, kind="ExternalInput")
    wpd = nc.dram_tensor("wpd", [128, HPC, HID], BF16, kind="ExternalInput")
    cosq = nc.dram_tensor("cosq", [128, S], BF16, kind="ExternalInput")
    sinq = nc.dram_tensor("sinq", [128, S], BF16, kind="ExternalInput")
    cosk = nc.dram_tensor("cosk", [128, S], BF16, kind="ExternalInput")
    sink = nc.dram_tensor("sink", [128, S], BF16, kind="ExternalInput")
    ones_col = nc.dram_tensor("ones_col", [128, 1], F32R, kind="ExternalInput")
    ones_bf = nc.dram_tensor("ones_bf", [128, 1], BF16, kind="ExternalInput")
    ones_row = nc.dram_tensor("ones_row", [1, 128], F32R, kind="ExternalInput")
    ident = nc.dram_tensor("ident", [128, 128], F32R, kind="ExternalInput")
    out = nc.dram_tensor("out", [S, HID], BF16, kind="ExternalOutput")
    if opt.get("debug"):
        dbg_q0 = nc.dram_tensor("dbg_q0", [128, S], BF16,
                                kind="ExternalOutput")
        dbg_k = nc.dram_tensor("dbg_k", [128, S], BF16, kind="ExternalOutput")
        dbg_v = nc.dram_tensor("dbg_v", [128, S], BF16, kind="ExternalOutput")
        dbg_g = nc.dram_tensor("dbg_g", [HPC, S], F32, kind="ExternalOutput")
        dbg_o0 = nc.dram_tensor("dbg_o0", [128, S], BF16,
                                kind="ExternalOutput")

    with tile.TileContext(nc, pool_alloc_mode=opt.get("palloc", "stack")) as tc:
      for _rep in range(repeat):
        with tc.tile_pool(name=f"persist{_rep}", bufs=1) as pers, \
             tc.tile_pool(name=f"scr{_rep}", bufs=1, space="DRAM") as dscr:
            # DRAM row scratch (partition reshapes / broadcasts)
            rrow_scr = dscr.tile([1, S], F32, name="rrow_scr")
            nk_scr = dscr.tile([1, S], F32, name="nk_scr")
            nq_scr = dscr.tile([HPC, S], F32, name="nq_scr")
            gate_scr = dscr.tile([HPC, S], F32, name="gate_scr")
            den_scr = dscr.tile([2 * HPC, 1024], F32, name="den_scr")

            # persistent tiles
            t_ones = pers.tile([128, 1], F32R, name="ones")
            nc.sync.dma_start(t_ones[:, :], ones_col[:, :])
            t_onesb = pers.tile([128, 1], BF16, name="onesb")
            nc.sync.dma_start(t_onesb[:, :], ones_bf[:, :])
            t_onesr = pers.tile([1, 128], F32R, name="onesr")
            nc.sync.dma_start(t_onesr[:, :], ones_row[:, :])
            t_id = pers.tile([128, 128], F32R, name="ident")
            nc.sync.dma_start(t_id[:, :], ident[:, :])

            q_t = [pers.tile([128, S], BF16, name=f"q{h}") for h in range(HPC)]
            k_t = pers.tile([128, S], BF16, name="kT")
            vnat = pers.tile([128, S], BF16, name="vnat")

            # ============ Stage 1: QKV + per-chunk post pipeline ============
            with tc.tile_pool(name=f"wq{_rep}", bufs=1) as wqp, \
                 tc.tile_pool(name=f"rt{_rep}", bufs=1) as rtp, \
                 tc.tile_pool(name=f"s1ps{_rep}", bufs=2, space="PSUM") as psB, \
                 tc.tile_pool(name=f"s1row{_rep}", bufs=5, space="PSUM") as psR, \
                 tc.tile_pool(name=f"s1tr{_rep}", bufs=1, space="PSUM") as psT, \
                 tc.tile_pool(name=f"s1x{_rep}", bufs=12) as sbx, \
                 tc.tile_pool(name=f"s1acc{_rep}", bufs=2) as sba, \
                 tc.tile_pool(name=f"s1sq{_rep}", bufs=2) as sbq, \
                 tc.tile_pool(name=f"s1d{_rep}", bufs=2) as sbd, \
                 tc.tile_pool(name=f"s1r{_rep}", bufs=2) as sbr:
                # resident weights, m-major (pieces stream in per m-group
                # during chunk 0)
                wq = wqp.tile([128, 8, HT, 128], BF16, name="wq")
                tcq = rtp.tile([128, S], BF16, name="tcq")
                tsq = rtp.tile([128, S], BF16, name="tsq")
                tck = rtp.tile([128, S], BF16, name="tck")
                tsk = rtp.tile([128, S], BF16, name="tsk")

                sq_state = {}

                def emit_qkv(ch):
                    """m-outer / ht-inner: each of the 8 output column
                    groups accumulates the full 5120 contraction into one
                    1-bank PSUM tile (ping-pong), drained right after its
                    stop. No 4-bank burst drain at chunk boundaries, so the
                    PE never idles long enough to re-throttle."""
                    c0 = ch * CW
                    xts = []
                    acc4 = sba.tile([128, 1024], F32, name="acc4")
                    for hq in range(HT // 4):
                        xt = sbx.tile([128, 4 * CW], BF16, name="xt")
                        nc.sync.dma_start(xt[:, :], xq[:, ch, hq, :])
                        xts.append(xt)
                        sqx = sbq.tile([128, 4 * CW], BF16, name="sqx")
                        with nc.allow_low_precision(
                                reason="x^2 in bf16; accr is bf16 anyway"):
                            nc.scalar.activation(sqx[:, :], xt[:, :],
                                                 AF.Square)
                        if hq == 0:
                            nc.gpsimd.tensor_copy(acc4[:, :], sqx[:, :])
                        else:
                            nc.gpsimd.tensor_add(acc4[:, :], acc4[:, :],
                                                 sqx[:, :])
                    vtmp = graw = None
                    for m in range(8):
                        if ch == 0:
                            # m-major weight piece; arrives while m-1's
                            # matmuls run. Scalar queue, so x tiles on SP
                            # aren't delayed.
                            nc.scalar.dma_start(wq[:, m, :, :],
                                                wqd[:, m, :, :])
                            if m == 6:
                                nc.scalar.dma_start(tcq[:, :], cosq[:, :])
                                nc.scalar.dma_start(tsq[:, :], sinq[:, :])
                                nc.scalar.dma_start(tck[:, :], cosk[:, :])
                                nc.scalar.dma_start(tsk[:, :], sink[:, :])
                        pmm = psB.tile([128, CW], F32, name="pmm")
                        for ht in range(HT):
                            nc.tensor.matmul(
                                pmm[:, :], wq[:, m, ht, :],
                                xts[ht // 4][:, (ht % 4) * CW:(ht % 4 + 1) * CW],
                                start=(ht == 0), stop=(ht == HT - 1))
                        if m == 0:
                            nc.vector.tensor_copy(k_t[:, c0:c0 + CW],
                                                  pmm[:, :])
                        elif m <= 5:
                            nc.vector.tensor_copy(q_t[m - 1][:, c0:c0 + CW],
                                                  pmm[:, :])
                        elif m == 6:
                            vtmp = sbd.tile([128, CW], F32R, name="vtmp")
                            nc.vector.tensor_copy(vtmp[:, :], pmm[:, :])
                        else:
                            graw = sbd.tile([5, CW], F32, name="graw")
                            nc.vector.tensor_copy(graw[:, :], pmm[0:5, :])
                        if m == 3 and ch >= 1:
                            # ch-1's stats/rope: its PE row matmuls slot in
                            # between m-groups, with Scalar (squares) and
                            # psR (5 bufs) comfortably ahead.
                            post_stats(ch - 1)
                    sq_state[ch] = [acc4, vtmp, graw]

                def post_squares(ch):
                    """Squares for the deferred stats matmuls. Emitted AFTER
                    post_stats(ch-1) so the stats Ln/Exp row ops sit ahead of
                    these in the Scalar queue (the row Lns free the psR psum
                    tiles the next stats matmuls need)."""
                    acc4, vtmp, graw = sq_state[ch]
                    c0 = ch * CW
                    accr = sbd.tile([128, CW], BF16, name="accr")
                    with nc.allow_low_precision(
                            reason="sum(x^2) ~ 5e3; bf16 rel err 4e-3 ok"):
                        nc.vector.tensor_reduce(
                            accr[:, :],
                            acc4[:, :].rearrange("p (q s) -> p s q", q=4),
                            axis=mybir.AxisListType.X, op=mybir.AluOpType.add)
                    ksq = sbd.tile([128, CW], BF16, name="ksq")
                    nc.scalar.activation(ksq[:, :], k_t[:, c0:c0 + CW],
                                         AF.Square)
                    qsq = sbd.tile([128, HPC, CW], BF16, name="qsq")
                    for h in range(HPC):
                        nc.scalar.activation(qsq[:, h, :],
                                             q_t[h][:, c0:c0 + CW], AF.Square)
                    sq_state[ch] = (accr, ksq, qsq, vtmp, graw)

                def post_stats(ch):
                    """Stats matmuls + rows + broadcasts + rope + v + gates."""
                    accr, ksq, qsq, vtmp, graw = sq_state.pop(ch)
                    c0 = ch * CW
                    # ---- stat rows (Act chain has no DMA round-trips) ----
                    pr = psR.tile([1, CW], F32, name="row")
                    nc.tensor.matmul(pr[:, :], t_onesb[:, :], accr[:, :],
                                     start=True, stop=True)
                    lnm_row = sbr.tile([1, CW], F32, name="lnm_row")
                    nc.scalar.activation(lnm_row[:, :], pr[:, :], AF.Ln,
                                         bias=EPS, scale=1.0 / HID)
                    r_row = sbr.tile([1, CW], F32, name="r_row")
                    nc.scalar.activation(r_row[:, :], lnm_row[:, :], AF.Exp,
                                         bias=0.0, scale=-0.5)
                    nc.sync.dma_start(rrow_scr[0:1, c0:c0 + CW], r_row[:, :])

                    pn = psR.tile([1, CW], F32, name="row")
                    nc.tensor.matmul(pn[:, :], t_onesb[:, :], ksq[:, :],
                                     start=True, stop=True)
                    lnk_row = sbr.tile([1, CW], F32, name="lnk_row")
                    nc.scalar.activation(lnk_row[:, :], pn[:, :], AF.Ln,
                                         bias=D * EPS, scale=1.0)
                    nkr = sbr.tile([1, CW], F32, name="nkr")
                    nc.scalar.activation(nkr[:, :], lnk_row[:, :], AF.Exp,
                                         bias=0.0, scale=-0.5)
                    nc.sync.dma_start(nk_scr[0:1, c0:c0 + CW], nkr[:, :])

                    for h in range(HPC):
                        pq = psR.tile([1, CW], F32, name="row")
                        nc.tensor.matmul(pq[:, :], t_onesb[:, :], qsq[:, h, :],
                                         start=True, stop=True)
                        lnq = sbr.tile([1, CW], F32, name="lnq")
                        nc.scalar.activation(lnq[:, :], pq[:, :], AF.Ln,
                                             bias=EPS, scale=1.0 / D)
                        nqr = sbr.tile([1, CW], F32, name="nqr")
                        nc.scalar.activation(nqr[:, :], lnq[:, :],
                                             AF.Exp, bias=0.0, scale=-0.5)
                        nc.sync.dma_start(nq_scr[h:h + 1, c0:c0 + CW],
                                          nqr[:, :])

                    # ---- broadcasts ----
                    rbig = sbr.tile([128, CW], F32, name="rbig")
                    nc.sync.dma_start(
                        rbig[:, :],
                        rrow_scr[0:1, c0:c0 + CW].to_broadcast((128, CW)))
                    nkb = sbr.tile([128, CW], F32, name="nkb")
                    nc.sync.dma_start(
                        nkb[:, :],
                        nk_scr[0:1, c0:c0 + CW].to_broadcast((128, CW)))
                    rb = sbr.tile([5, CW], F32, name="rb")
                    nc.sync.dma_start(
                        rb[:, :],
                        rrow_scr[0:1, c0:c0 + CW].to_broadcast((5, CW)))

                    # ---- v: scale by r then transpose ----
                    nc.vector.tensor_mul(vtmp[:, :], vtmp[:, :], rbig[:, :])
                    for j in range(2):
                        kt = 2 * ch + j
                        ptr = psT.tile([128, 128], F32R, name="tr")
                        nc.tensor.transpose(ptr[:, :],
                                            vtmp[:, j * 128:(j + 1) * 128],
                                            t_id[:, :])
                        nc.vector.tensor_copy(
                            vnat[:, kt * 128:(kt + 1) * 128], ptr[:, :])

                    # ---- k rope + fold k-norm (exp scale becomes 1) ----
                    rot = sbr.tile([128, CW], BF16, name="rotk")
                    nc.sync.dma_start(rot[0:64, :], k_t[64:128, c0:c0 + CW])
                    nc.sync.dma_start(rot[64:128, :], k_t[0:64, c0:c0 + CW])
                    t1 = sbr.tile([128, CW], F32, name="t1k")
                    nc.vector.tensor_mul(t1[:, :], k_t[:, c0:c0 + CW],
                                         tck[:, c0:c0 + CW])
                    t2 = sbr.tile([128, CW], F32, name="t2k")
                    nc.vector.tensor_mul(t2[:, :], rot[:, :],
                                         tsk[:, c0:c0 + CW])
                    nc.vector.tensor_add(t1[:, :], t1[:, :], t2[:, :])
                    nc.vector.tensor_mul(k_t[:, c0:c0 + CW], t1[:, :],
                                         nkb[:, :])

                    # ---- q rope per head ----
                    for h in range(HPC):
                        nb = sbr.tile([128, CW], F32, name="nb")
                        nc.sync.dma_start(
                            nb[:, :],
                            nq_scr[h:h + 1, c0:c0 + CW].to_broadcast((128, CW)))
                        rotq = sbr.tile([128, CW], BF16, name="rotq")
                        nc.sync.dma_start(rotq[0:64, :],
                                          q_t[h][64:128, c0:c0 + CW])
                        nc.sync.dma_start(rotq[64:128, :],
                                          q_t[h][0:64, c0:c0 + CW])
                        tq1 = sbr.tile([128, CW], F32, name="tq1")
                        nc.vector.tensor_mul(tq1[:, :], q_t[h][:, c0:c0 + CW],
                                             tcq[:, c0:c0 + CW])
                        tq2 = sbr.tile([128, CW], F32, name="tq2")
                        nc.vector.tensor_mul(tq2[:, :], rotq[:, :],
                                             tsq[:, c0:c0 + CW])
                        nc.vector.tensor_add(tq1[:, :], tq1[:, :], tq2[:, :])
                        nc.vector.tensor_mul(q_t[h][:, c0:c0 + CW], tq1[:, :],
                                             nb[:, :])

                    # ---- gates: store softplus(-x) = -ln(sigmoid(x)).
                    # The tail computes exp(-(ln denom + sp)) = gate/denom,
                    # so no reciprocal is ever needed (DVE reciprocal is
                    # 8 cycles/element -- 2.1us per [5,CW] row).
                    nc.vector.tensor_mul(graw[:, :], graw[:, :], rb[:, :])
                    ge = sbr.tile([5, CW], F32, name="ge")
                    nc.scalar.activation(ge[:, :], graw[:, :], AF.Exp,
                                         bias=0.0, scale=-1.0)
                    nc.vector.tensor_scalar_add(ge[:, :], ge[:, :], 1.0)
                    sp = sbr.tile([5, CW], F32, name="sp")
                    nc.scalar.activation(sp[:, :], ge[:, :], AF.Ln)
                    nc.sync.dma_start(gate_scr[0:5, c0:c0 + CW], sp[:, :])

                for ch in range(NCH):
                    emit_qkv(ch)
                    post_squares(ch)
                post_stats(NCH - 1)

            if opt.get("debug"):
                nc.sync.dma_start(dbg_q0[:, :], q_t[0][:, :])
                nc.sync.dma_start(dbg_k[:, :], k_t[:, :])
                nc.sync.dma_start(dbg_v[:, :], vnat[:, :])
                nc.sync.dma_start(dbg_g[:, :], gate_scr[:, :])

            # ============ Stages 2+3 ============
            with tc.tile_pool(name=f"wp{_rep}", bufs=1) as wpp:
                # prefetch proj weights while attention runs
                wp = wpp.tile([128, HPC, HID], BF16, name="wp")
                nc.sync.dma_start(wp[:, :, :], wpd[:, :, :])
                o_t = [wpp.tile([128, S], BF16, name=f"o{h}")
                       for h in range(HPC)]

                # ---- Stage 2: attention ----
                with tc.tile_pool(name=f"at_sc{_rep}", bufs=2, space="PSUM") as pSC, \
                     tc.tile_pool(name=f"at_av{_rep}", bufs=1, space="PSUM") as pAV, \
                     tc.tile_pool(name=f"at_row{_rep}", bufs=1, space="PSUM") as pRow, \
                     tc.tile_pool(name=f"at_acc{_rep}", bufs=1) as asb1, \
                     tc.tile_pool(name=f"at_sb{_rep}", bufs=3) as asb2, \
                     tc.tile_pool(name=f"at_et{_rep}", bufs=5) as asb3:
                    pending_tail = []

                    def emit_core(h, qp):
                        c0 = qp * 1024
                        po = pAV.tile([128, 1024], F32, name="po")
                        accA = asb1.tile([128, 1024], BF16, name="accA")
                        accB = asb1.tile([128, 1024], BF16, name="accB")
                        accC = asb1.tile([128, 1024], BF16, name="accC",
                                         bufs=2)
                        ps_tiles = {}

                        def emit_sc(kt):
                            ps = pSC.tile([128, 1024], F32, name="sc")
                            for j in range(2):
                                nc.tensor.matmul(
                                    ps[:, j * 512:(j + 1) * 512],
                                    k_t[:, kt * 128:(kt + 1) * 128],
                                    q_t[h][:, c0 + j * 512:c0 + (j + 1) * 512],
                                    start=True, stop=True)
                            ps_tiles[kt] = ps

                        emit_sc(0)
                        for kt in range(KT):
                            k0 = kt * 128
                            if kt + 1 < KT:
                                emit_sc(kt + 1)
                            ps = ps_tiles.pop(kt)
                            et = asb3.tile([128, 1024], BF16, name="expt")
                            nc.scalar.activation(et[:, :], ps[:, :], AF.Exp)
                            for j in range(2):
                                nc.tensor.matmul(
                                    po[:, j * 512:(j + 1) * 512],
                                    vnat[:, k0:k0 + 128],
                                    et[:, j * 512:(j + 1) * 512],
                                    start=(kt == 0), stop=(kt == KT - 1))
                            use_pool = h > 0 and (kt == 0 or kt % 4 == 0)
                            with nc.allow_low_precision(
                                    reason="softmax denom; 2e-2 tolerance"):
                                if kt == 0 and use_pool:
                                    nc.gpsimd.tensor_copy(accA[:, :], et[:, :])
                                elif kt == 0:
                                    nc.vector.tensor_copy(accA[:, :], et[:, :])
                                elif kt == 1:
                                    nc.vector.tensor_copy(accB[:, :], et[:, :])
                                elif use_pool:
                                    nc.gpsimd.tensor_add(accA[:, :],
                                                         accA[:, :], et[:, :])
                                else:
                                    nc.vector.tensor_add(accB[:, :],
                                                         accB[:, :], et[:, :])
                        with nc.allow_low_precision(
                                reason="softmax denom; 2e-2 tolerance"):
                            nc.vector.tensor_add(accC[:, :], accA[:, :],
                                                 accB[:, :])
                        # drain AV psum to o_t (unscaled); split engines so
                        # both po banks free in parallel
                        nc.vector.tensor_copy(o_t[h][:, c0:c0 + 512],
                                              po[:, 0:512])
                        nc.scalar.copy(o_t[h][:, c0 + 512:c0 + 1024],
                                       po[:, 512:1024])
                        return accC

                    def emit_tail(h, qp, accC):
                        # o_t scale = gate/denom per q column. Row reduce on
                        # PE, ln on the Scalar row pipe (rows are full speed
                        # on ACT, ~6x slower per element on DVE), broadcast
                        # via DRAM round-trip, exp(-x) full-width, then two
                        # DVE muls. Deferred one block: nothing here stalls
                        # the PE queue.
                        c0 = qp * 1024
                        b = h * 2 + qp
                        prow = pRow.tile([1, 1024], F32, name="drow")
                        for j in range(2):
                            nc.tensor.matmul(prow[0:1, j * 512:(j + 1) * 512],
                                             t_onesb[:, :],
                                             accC[:, j * 512:(j + 1) * 512],
                                             start=True, stop=True)
                        lnr = asb2.tile([1, 1024], F32, name="lnr", bufs=2)
                        nc.scalar.activation(lnr[:, :], prow[0:1, :], AF.Ln)
                        nc.sync.dma_start(den_scr[b:b + 1, :], lnr[:, :])
                        lnb = asb2.tile([128, 1024], F32, name="lnb", bufs=2)
                        nc.sync.dma_start(
                            lnb[:, :],
                            den_scr[b:b + 1, :].to_broadcast((128, 1024)))
                        gab = asb2.tile([128, 1024], F32, name="gab", bufs=2)
                        nc.sync.dma_start(
                            gab[:, :],
                            gate_scr[h:h + 1, c0:c0 + 1024].to_broadcast(
                                (128, 1024)))
                        # gate_scr holds -ln(gate); exp(-(ln den + sp))
                        # = gate/denom in one activation
                        rcb = asb2.tile([128, 1024], F32, name="rcb", bufs=2)
                        nc.vector.tensor_add(lnb[:, :], lnb[:, :], gab[:, :])
                        nc.scalar.activation(rcb[:, :], lnb[:, :], AF.Exp,
                                             bias=0.0, scale=-1.0)
                        nc.vector.tensor_mul(o_t[h][:, c0:c0 + 1024],
                                             o_t[h][:, c0:c0 + 1024],
                                             rcb[:, :])

                    for h in range(HPC):
                        for qp in range(2):
                            accC = emit_core(h, qp)
                            pending_tail.append((h, qp, accC))
                            if len(pending_tail) > 1:
                                emit_tail(*pending_tail.pop(0))
                    while pending_tail:
                        emit_tail(*pending_tail.pop(0))

                if opt.get("debug"):
                    nc.sync.dma_start(dbg_o0[:, :], o_t[0][:, :])

                # ---- Stage 3: projection ----
                with tc.tile_pool(name=f"pj_ps{_rep}", bufs=3, space="PSUM") as pPJ, \
                     tc.tile_pool(name=f"pj_sb{_rep}", bufs=3) as pjs:
                    NTP = HID // 1024  # 5
                    for ntp in range(NTP):
                        n0 = ntp * 1024
                        for st in range(KT):
                            s0 = st * 128
                            pp = pPJ.tile([128, 1024], F32, name="pj")
                            # h outer / j inner: consecutive matmul pairs
                            # share the stationary (one LDWEIGHTS per pair)
                            # and alternate PSUM banks.
                            for h in range(HPC):
                                for j in range(2):
                                    nc.tensor.matmul(
                                        pp[:, j * 512:(j + 1) * 512],
                                        o_t[h][:, s0:s0 + 128],
                                        wp[:, h, n0 + j * 512:n0 + (j + 1) * 512],
                                        start=(h == 0), stop=(h == HPC - 1))
                            ob = pjs.tile([128, 1024], BF16, name="outsb")
                            if st % 2 == 0:
                                nc.vector.tensor_copy(ob[:, :], pp[:, :])
                            else:
                                nc.scalar.copy(ob[:, :], pp[:, :])
                            nc.sync.dma_start(out[s0:s0 + 128, n0:n0 + 1024],
                                              ob[:, :])
    nc.finalize()
    return nc


# ---------------- host-side prep & execution ----------------

_CACHE = {}


def _get_exec(repeat=1):
    key = (repeat, tuple(sorted(BUILD_OPTS.items())))
    if key in _CACHE:
        return _CACHE[key]

    import jax
    from concourse import bass2jax, mybir as mb
    from jax.experimental.shard_map import shard_map
    from jax.sharding import Mesh, PartitionSpec

    bass2jax.install_neuronx_cc_hook()
    nc = build_program(repeat)

    part_name = nc.partition_id_tensor.name if nc.partition_id_tensor else None
    in_names, out_names, out_avals = [], [], []
    for alloc in nc.m.functions[0].allocations:
        if not isinstance(alloc, mb.MemoryLocationSet):
            continue
        name = alloc.memorylocations[0].name
        if alloc.kind == "ExternalInput":
            if name != part_name:
                in_names.append(name)
        elif alloc.kind == "ExternalOutput":
            out_names.append(name)
            out_avals.append(jax.core.ShapedArray(tuple(alloc.tensor_shape),
                                                  mb.dt.np(alloc.dtype)))
    n_params = len(in_names)
    all_names = in_names + out_names
    if part_name is not None:
        all_names = all_names + [part_name]

    def _body(*args):
        operands = list(args)
        if part_name is not None:
            operands.append(bass2jax.partition_id_tensor())
        outs = bass2jax._bass_exec_p.bind(
            *operands,
            out_avals=tuple(out_avals),
            in_names=tuple(all_names),
            out_names=tuple(out_names),
            lowering_input_output_aliases=(),
            sim_require_finite=True,
            sim_require_nnan=True,
            nc=nc,
        )
        return tuple(outs)

    devices = jax.devices()[:NC]
    mesh = Mesh(np.asarray(devices), ("core",))
    spec = (PartitionSpec("core"),) * (n_params + len(out_names))
    # Donate the output-backing buffers: without donation XLA copies the
    # zero-init arrays into fresh output buffers every dispatch (~21MB/core).
    donate = tuple(range(n_params, n_params + len(out_names)))
    fn = jax.jit(shard_map(_body, mesh=mesh, in_specs=spec,
                           out_specs=(PartitionSpec("core"),) * len(out_names),
                           check_rep=False), keep_unused=True,
                 donate_argnums=donate)
    _CACHE[key] = dict(fn=fn, nc=nc, in_names=in_names, out_names=out_names,
                       out_avals=out_avals, mesh=mesh)
    return _CACHE[key]


def prep_inputs(x, rope_cos, rope_sin, w_pre_norm, w_qkv, w_q_norm, w_k_norm,
                w_proj):
    """Build the per-core input dict list (host-side sharding/layout only)."""
    import ml_dtypes
    bf16 = ml_dtypes.bfloat16

    x = np.asarray(x, np.float32)
    w_qkv = np.asarray(w_qkv, np.float32)
    w_proj = np.asarray(w_proj, np.float32)
    w_pre = np.asarray(w_pre_norm, np.float32)
    w_qn = np.asarray(w_q_norm, np.float32)
    w_kn = np.asarray(w_k_norm, np.float32)
    cos = np.asarray(rope_cos, np.float32)[0]   # [S, D]
    sin = np.asarray(rope_sin, np.float32)[0]

    # x: [S, HID] -> [128, NCH, HT//4, 4*CW] bf16 (hq-group contiguous)
    xT = x[0].T                                  # [HID, S]
    xqh = np.ascontiguousarray(
        xT.reshape(HT // 4, 4, 128, NCH, CW).transpose(2, 3, 0, 1, 4)
        .reshape(128, NCH, HT // 4, 4 * CW)).astype(bf16)

    cosT = np.ascontiguousarray(cos.T)          # [D, S]
    sinT = np.ascontiguousarray(sin.T)
    sign = np.where(np.arange(D) < D // 2, -1.0, 1.0).astype(np.float32)

    def rope_tables(w):
        w_swap = np.concatenate([w[D // 2:], w[:D // 2]])
        c = cosT * w[:, None]
        s = sinT * (sign * w_swap)[:, None]
        return np.ascontiguousarray(c), np.ascontiguousarray(s)

    cq, sq_ = rope_tables(w_qn)
    ck, sk = rope_tables(w_kn)
    cq, sq_, ck, sk = (a.astype(bf16) for a in (cq, sq_, ck, sk))

    wqkv_eff = w_pre[:, None] * w_qkv           # fold pre-norm weight (exact)
    q_dim, k_dim = NQ * D, NKV * D
    ones = np.ones((128, 1), np.float32)
    ones_b = np.ones((128, 1), bf16)
    ones_r = np.ones((1, 128), np.float32)
    ident = np.eye(128, dtype=np.float32)

    gate_pad = np.zeros((HID, 128 - HPC), np.float32)
    in_maps = []
    for c in range(NC):
        wslice = np.concatenate([
            wqkv_eff[:, q_dim + c * D:q_dim + (c + 1) * D],
            wqkv_eff[:, (HPC * c) * D:(HPC * c + HPC) * D],
            wqkv_eff[:, q_dim + k_dim + c * D:q_dim + k_dim + (c + 1) * D],
            wqkv_eff[:, q_dim + 2 * k_dim + HPC * c:q_dim + 2 * k_dim + HPC * (c + 1)],
            gate_pad,
        ], axis=1)                               # [HID, 1024] (k first)
        # [HID, 1024] -> [128, 8, HT, 128] m-major
        wqd = np.ascontiguousarray(
            wslice.reshape(HT, 128, 8, 128).transpose(1, 2, 0, 3)).astype(bf16)
        wpd = np.ascontiguousarray(
            w_proj[(HPC * c) * D:(HPC * c + HPC) * D, :].reshape(
                HPC, 128, HID).transpose(1, 0, 2)).astype(bf16)
        in_maps.append({
            "xq": xqh, "wqd": wqd, "wpd": wpd,
            "cosq": cq, "sinq": sq_, "cosk": ck, "sink": sk,
            "ones_col": ones, "ones_bf": ones_b, "ones_row": ones_r,
            "ident": ident,
        })
    return in_maps


def run_in_maps(in_maps):
    """Execute the SPMD program; returns list of per-core {out: [S, HID]}."""
    cache = _get_exec()
    fn, in_names, out_names, out_avals = (cache["fn"], cache["in_names"],
                                          cache["out_names"], cache["out_avals"])
    concat_in = [np.concatenate([m[nm] for m in in_maps], axis=0)
                 for nm in in_names]
    zeros = [np.zeros((NC * a.shape[0], *a.shape[1:]), a.dtype) for a in out_avals]
    outs = fn(*concat_in, *zeros)
    res = []
    for c in range(NC):
        d = {}
        for i, nm in enumerate(out_names):
            shp = out_avals[i].shape
            d[nm] = np.asarray(outs[i]).reshape(NC, *shp)[c]
        res.append(d)
    return res


def kernel(**inputs):
    in_maps = prep_inputs(**inputs)
    res = run_in_maps(in_maps)
    total = res[0]["out"].astype(np.float32)
    for c in range(1, NC):
        total = total + res[c]["out"].astype(np.float32)
    return total.reshape(1, S, HID)

